# revision 31
# baseline (speedup 1.0000x reference)
"""Multi-head causal attention (B=4, S=2048, D=1024, H=16) on 8 Trainium2
NeuronCores.

Sharding: core c handles batch c//2 and head-group c%2 (8 of 16 heads).
QKV weights are column-sharded per head-group; attention runs fully local.
Context vectors (bf16) are AllGathered pairwise in per-head-pair chunks
(the last chunk split by token range) so gathers overlap attention; each
core applies a column shard of the output projection.  Wo input rows are
pre-shuffled on the host to match the chunked gather's row order.

Single software-pipelined emission: projections for head-pair hp+1 and
output-projection partial sums for hp-1 are interleaved into hp's
attention inner loop, keeping PE busy under the ACT-bound exp stream.
The ctx matmuls lag the score matmuls by one group so the in-order PE
queue never stalls waiting on exp.  Scores for the even/odd head of a
pair are emitted adjacently with 64-partition operands so the PE
row-tiles them concurrently on HW.  Causal masking is applied post-exp
with gpsimd affine_select (zero fill) - no mask matmuls or bias tensors.
"""

from collections import deque

import numpy as np

import concourse.bass as bass
import concourse.tile as tile
from concourse import bacc, mybir
from concourse.bass import ts
from concourse.bass_utils import run_bass_kernel_spmd
from concourse.masks import make_identity

B, S, D, H, HD = 4, 2048, 1024, 16, 64
P = 128
DPC = 512                 # q/k/v dims per core (8 heads)
NT = S // P               # 16 token chunks
NKO = D // P              # 8 contraction chunks of the model dim
NQ = S // 512             # 4 q chunks of 512
NHP = DPC // P            # 4 local head pairs
F32 = mybir.dt.float32
FR = mybir.dt.float32r
BF16 = mybir.dt.bfloat16
EXP = mybir.ActivationFunctionType.Exp
MUL = mybir.AluOpType.mult
ADD = mybir.AluOpType.add
GROUPS = [[0, 1], [2, 3], [4, 5], [6, 7]]
# per head-pair gather pieces: (token offset, length, staged-after-chunk)
# stage=None means staged at the end of the head-pair's attention
GATHER_PIECES = {
    0: [(0, S, None)],
    1: [(0, S, None)],
    2: [(0, 1024, 1), (1024, 1024, None)],
    3: [(0, 1024, 1), (1024, 1024, None)],
}

_CACHE = {}


def build_nc():
    nc = bacc.Bacc("TRN2", target_bir_lowering=False, debug=False, num_devices=8)

    x_d = nc.declare_dram_parameter("x", [S, D], BF16, isOutput=False)
    wq_d = nc.declare_dram_parameter("wq", [D, DPC], BF16, isOutput=False)
    wk_d = nc.declare_dram_parameter("wk", [D, DPC], BF16, isOutput=False)
    wv_d = nc.declare_dram_parameter("wv", [D, DPC], BF16, isOutput=False)
    wo_d = nc.declare_dram_parameter("wo", [D, DPC], BF16, isOutput=False)
    bo_d = nc.declare_dram_parameter("bo", [P, DPC], F32, isOutput=False)
    out_d = nc.declare_dram_parameter("out", [S, DPC], F32, isOutput=True)

    with tile.TileContext(nc) as tc:
        with (
            tc.tile_pool(name="const", bufs=1) as cst,
            tc.tile_pool(name="big", bufs=1) as big,
            tc.tile_pool(name="dram", bufs=1, space="DRAM") as dramp,
            tc.tile_pool(name="cp", bufs=1) as cp,
            tc.tile_pool(name="ep", bufs=2) as ep,
            tc.tile_pool(name="xst", bufs=2) as xst,
            tc.tile_pool(name="cxf", bufs=2) as cxf,
            tc.tile_pool(name="cxd", bufs=2) as cxd,
            tc.tile_pool(name="psS", bufs=1, space="PSUM") as pss,
            tc.tile_pool(name="psC", bufs=1, space="PSUM") as psc,
            tc.tile_pool(name="psX", bufs=2, space="PSUM") as psx,
        ):
            ident = cst.tile([P, P], BF16)
            make_identity(nc, ident[:])
            ones_f = cst.tile([P, 64], F32)
            nc.vector.memset(ones_f[:], 1.0)
            ones_fr = cst.tile([P, 64], FR)
            nc.vector.tensor_copy(ones_fr[:], ones_f[:])
            bo_sb = cst.tile([P, DPC], F32)
            nc.sync.dma_start(bo_sb[:], bo_d[:])

            # persistent intermediates
            xT = big.tile([P, NKO, S], BF16)
            qT = big.tile([P, NHP, S], BF16)       # [dh, pair, tok]
            kT = big.tile([P, NHP, S], BF16)
            v_sb = big.tile([P, NT, 8, 65], BF16)  # [tok, chunk, head, dh+1]
            acc = big.tile([P, NT, DPC], F32)      # out-proj accumulator
            wk_sb = big.tile([P, NKO, DPC], BF16)
            wq_sb = big.tile([P, NKO, DPC], BF16)
            wv_sb = big.tile([P, NKO, DPC], BF16)
            wo_sb = big.tile([P, NKO, DPC], BF16)
            nc.gpsimd.dma_start(wv_sb[:], wv_d.rearrange("(ko p) n -> p ko n", p=P))
            nc.gpsimd.dma_start(wk_sb[:], wk_d.rearrange("(ko p) n -> p ko n", p=P))
            nc.gpsimd.dma_start(wq_sb[:], wq_d.rearrange("(ko p) n -> p ko n", p=P))
            nc.gpsimd.dma_start(wo_sb[:], wo_d.rearrange("(ko p) n -> p ko n", p=P))
            nc.vector.memset(v_sb[:, :, :, 64:65], 1.0)

            ctx_loc, ctx_ful = {}, {}
            for hp, pieces in GATHER_PIECES.items():
                for pi, (off, ln, _) in enumerate(pieces):
                    ctx_loc[(hp, pi)] = dramp.tile(
                        [P, ln], BF16, name=f"ctx_loc{hp}_{pi}")
                    ctx_ful[(hp, pi)] = dramp.tile(
                        [2 * P, ln], BF16, name=f"ctx_ful{hp}_{pi}")

            # ---- emission helpers ----
            def v_unit(t):
                def emit():
                    pv = psx.tile([P, DPC], F32, tag="x", name="pv")
                    for ko in range(NKO):
                        nc.tensor.matmul(
                            pv[:],
                            xT[:, ko, ts(t, P)],
                            wv_sb[:, ko, :],
                            start=(ko == 0),
                            stop=(ko == NKO - 1),
                        )
                    nc.vector.tensor_copy(
                        v_sb[:, t, :, 0:64],
                        pv[:].rearrange("p (h d) -> p h d", h=8),
                    )
                return emit

            def proj_unit(w_sb, outT, hp, n):
                def emit():
                    pq = psx.tile([P, DPC], F32, tag="x", name="pq")
                    for ko in range(NKO):
                        nc.tensor.matmul(
                            pq[:],
                            w_sb[:, ko, ts(hp, P)],
                            xT[:, ko, ts(n, 512)],
                            start=(ko == 0),
                            stop=(ko == NKO - 1),
                        )
                    nc.vector.tensor_copy(outT[:, hp, ts(n, 512)], pq[:])
                return emit

            def po_unit(hp, t, ctxf, t_off, first):
                def emit():
                    po = psx.tile([P, DPC], F32, tag="x", name="po")
                    for par in range(2):
                        nc.tensor.matmul(
                            po[:],
                            ctxf[:, par, ts(t - t_off, P)],
                            wo_sb[:, 2 * hp + par, :],
                            start=(par == 0),
                            stop=(par == 1),
                        )
                    if first:
                        nc.vector.tensor_tensor(acc[:, t, :], po[:], bo_sb[:], ADD)
                    else:
                        nc.vector.tensor_tensor(
                            acc[:, t, :], acc[:, t, :], po[:], ADD
                        )
                return emit

            pendn = deque()

            def normalize(pctx, ctx_dst, h01, c):
                # stage 1: reciprocal of the denominator row (DVE only)
                def stage1():
                    rec = cp.tile([P, 512], FR, tag=f"rec{h01}", name="rec")
                    with nc.allow_low_precision(reason="softmax recip"):
                        nc.vector.reciprocal(rec[64:65, :], pctx[64:65, :])

                    # stage 2: rank-1 broadcast via PE + scale (rec is ready)
                    def stage2():
                        pscl = psx.tile([P, DPC], F32, tag="x", name="pscl")
                        nc.tensor.matmul(
                            pscl[0:64, :], ones_fr[64:65, :], rec[64:65, :],
                            start=True, stop=True,
                        )
                        scl = cp.tile([64, 512], F32, tag=f"scl{h01}", name="scl")
                        nc.vector.tensor_copy(scl[:], pscl[0:64, :])
                        if h01 == 0:
                            nc.vector.tensor_tensor(
                                ctx_dst[0:64, ts(c, 512)], pctx[0:64, :],
                                scl[:], MUL,
                            )
                        else:
                            tmp = cp.tile([64, 512], BF16, tag="tmp", name="tmp")
                            nc.vector.tensor_tensor(
                                tmp[:], pctx[0:64, :], scl[:], MUL
                            )
                            nc.sync.dma_start(ctx_dst[64:128, ts(c, 512)], tmp[:])

                    pendn.append(stage2)
                return stage1

            def stage_piece(hp, pi, ctx_dst):
                off, ln, _ = GATHER_PIECES[hp][pi]
                nc.sync.dma_start(
                    ctx_loc[(hp, pi)][:], ctx_dst[:, off:off + ln]
                )
                nc.gpsimd.collective_compute(
                    "AllGather",
                    mybir.AluOpType.bypass,
                    replica_groups=GROUPS,
                    ins=[ctx_loc[(hp, pi)][:]],
                    outs=[ctx_ful[(hp, pi)][:]],
                )

            def attn_steps(hp, out):
                ctx_dst = cxd.tile([P, S], BF16, tag="ctxd", name=f"ctxd{hp}")
                out["ctx_dst"] = ctx_dst
                es = {}
                pctx_of = {}
                prev = []

                def ctx_mms(c, g):
                    nkb = 4 * c + 4
                    pctx = pctx_of[c]
                    for h01 in range(2):
                        e = es.pop((h01, g))
                        for dm in range(2):
                            m = 2 * g + dm
                            nc.tensor.matmul(
                                pctx[h01][0:65, :],
                                v_sb[:, m, 2 * hp + h01, 0:65],
                                e[:, dm, :],
                                start=(m == 0),
                                stop=(m == nkb - 1),
                            )

                def chunk_close(c):
                    ctx_mms(c, 2 * c + 1)
                    normalize(pctx_of[c][0], ctx_dst, 0, c)()
                    normalize(pctx_of[c][1], ctx_dst, 1, c)()

                staged = [pi for pi, p in enumerate(GATHER_PIECES[hp])
                          if p[2] is not None]
                for c in range(NQ):
                    pctx_of[c] = (
                        psc.tile([P, 512], F32, tag="c0", name="pctxE"),
                        psc.tile([P, 512], F32, tag="c1", name="pctxO"),
                    )
                    ngr = 2 * c + 2
                    for g in range(ngr):
                        yield c
                        sg = [
                            pss.tile([P, 2, 512], F32, tag=f"s{h}",
                                     name=f"sg{h}")
                            for h in range(2)
                        ]
                        # adjacent even/odd-head matmuls -> concurrent
                        # row-group tiles on HW (K=64 each)
                        for dm in range(2):
                            m = 2 * g + dm
                            for h01 in range(2):
                                off = 64 * h01
                                nc.tensor.matmul(
                                    sg[h01][:, dm, :],
                                    kT[off:off + 64, hp, ts(m, P)],
                                    qT[off:off + 64, hp, ts(c, 512)],
                                    start=True,
                                    stop=True,
                                )
                        for h01 in range(2):
                            e = ep.tile([P, 2, 512], BF16, tag=f"e{h01}",
                                        name=f"e{h01}")
                            es[(h01, g)] = e
                            nc.scalar.activation(
                                e[:], sg[h01][:], EXP, scale=0.125
                            )
                            # causal mask: zero e where key > query
                            for dm in range(2):
                                m = 2 * g + dm
                                dd = m - 4 * c
                                if dd >= 0:
                                    w = 128 * (dd + 1)
                                    nc.gpsimd.affine_select(
                                        out=e[:, dm, 0:w],
                                        in_=e[:, dm, 0:w],
                                        compare_op=mybir.AluOpType.is_ge,
                                        fill=0.0,
                                        base=-128 * dd,
                                        pattern=[[1, w]],
                                        channel_multiplier=-1,
                                    )
                        # deferred work rides behind the score/exp feed
                        for fn in prev:
                            fn()
                        prev = []
                        if g == 1 and staged and \
                                GATHER_PIECES[hp][staged[0]][2] == c - 1:
                            while pendn:
                                pendn.popleft()()
                            stage_piece(hp, staged.pop(0), ctx_dst)
                        drain(pendn, 2)
                        drain(pend_pe, 2)
                        if g == ngr - 1:
                            prev.append(lambda c=c: chunk_close(c))
                        else:
                            prev.append(lambda c=c, g=g: ctx_mms(c, g))
                for fn in prev:
                    fn()
                while pendn:
                    pendn.popleft()()

            # ---- phase A: transpose x; v + kq0 + early attn0 ride along ----
            pend_pe = deque()
            ctxf_tiles = {}

            def drain(q, n):
                for _ in range(min(n, len(q))):
                    q.popleft()()

            out_of = {0: {}}
            gen0 = attn_steps(0, out_of[0])
            next_c = [next(gen0)]

            def pull0(limit_c):
                if next_c[0] is not None and next_c[0] <= limit_c:
                    try:
                        next_c[0] = next(gen0)
                    except StopIteration:
                        next_c[0] = None
                    return True
                return False

            with nc.named_scope("phaseA"):
                for t in range(NT):
                    x_st = xst.tile([P, D], BF16, tag="x")
                    nc.sync.dma_start(
                        x_st[:], x_d.rearrange("(t p) d -> p t d", p=P)[:, t, :]
                    )
                    for half in range(2):
                        pt = psx.tile([P, 4, P], BF16, tag="x", name="pt")
                        for q in range(4):
                            nc.tensor.transpose(
                                pt[:, q, :], x_st[:, ts(4 * half + q, P)], ident[:]
                            )
                        if half == 1 and t > 0:
                            v_unit(t - 1)()
                            if t % 4 == 0:
                                n = t // 4 - 1
                                proj_unit(wk_sb, kT, 0, n)()
                                proj_unit(wq_sb, qT, 0, n)()
                        nc.vector.tensor_copy(
                            xT[:, 4 * half:4 * half + 4, ts(t, P)], pt[:]
                        )
                    if t >= 4:
                        pull0((t - 4) // 4)
                        if t >= 12:
                            pull0((t - 4) // 4)
                v_unit(NT - 1)()
                proj_unit(wk_sb, kT, 0, NQ - 1)()
                proj_unit(wq_sb, qT, 0, NQ - 1)()

            # ---- main interleaved loop ----
            for hp in range(NHP):
                with nc.named_scope(f"attn{hp}"):
                    if hp < NHP - 1:
                        for n in range(NQ):
                            pend_pe.append(proj_unit(wk_sb, kT, hp + 1, n))
                            pend_pe.append(proj_unit(wq_sb, qT, hp + 1, n))
                    if hp == 0:
                        while pull0(NQ):
                            pass
                    else:
                        out_of[hp] = {}
                        for _ in attn_steps(hp, out_of[hp]):
                            pass
                    ctx_dst = out_of[hp]["ctx_dst"]
                    drain(pend_pe, len(pend_pe))
                    if hp >= 1:
                        # gate: po batch for hp-1 cannot be scheduled before
                        # this attention's last ctx chunk is written
                        for pi, (off, ln, _) in enumerate(GATHER_PIECES[hp - 1]):
                            tag = ("cfS" if ln == S
                                   else f"cf{hp - 1}_{pi}")
                            cf = cxf.tile([P, 2, ln], BF16, tag=tag,
                                          name=f"ctxf{hp - 1}_{pi}", bufs=1)
                            nc.vector.tensor_copy(
                                cf[0:1, 0:1, 0:2], ctx_dst[0:1, S - 2:S]
                            )
                            nc.gpsimd.dma_start(
                                cf[:],
                                ctx_ful[(hp - 1, pi)].rearrange(
                                    "(h p) t -> p h t", p=P),
                            )
                            t0 = off // P
                            for t in range(t0, t0 + ln // P):
                                pend_pe.append(
                                    po_unit(hp - 1, t, cf, t0,
                                            first=(hp - 1 == 0))
                                )
                    # stage remaining (end) pieces for this hp
                    for pi, (off, ln, stage_c) in enumerate(GATHER_PIECES[hp]):
                        if stage_c is None:
                            stage_piece(hp, pi, ctx_dst)

            # ---- tail: hp3 out-proj pieces + write out ----
            with nc.named_scope("tail"):
                drain(pend_pe, len(pend_pe))
                ctxd3 = out_of[3]["ctx_dst"]
                out_r = out_d.rearrange("(t p) n -> p t n", p=P)
                gate_col = {0: S, 1: S}
                for pi, (off, ln, _) in enumerate(GATHER_PIECES[3]):
                    cf = cxf.tile([P, 2, ln], BF16, tag=f"cf3_{pi}",
                                  name=f"ctxf3_{pi}", bufs=1)
                    gc = gate_col[pi]
                    nc.vector.tensor_copy(
                        cf[0:1, 0:1, 0:2], ctxd3[0:1, gc - 2:gc]
                    )
                    nc.gpsimd.dma_start(
                        cf[:],
                        ctx_ful[(3, pi)].rearrange("(h p) t -> p h t", p=P),
                    )
                    t0 = off // P
                    for t in range(t0, t0 + ln // P):
                        po_unit(3, t, cf, t0, first=False)()
                        nc.sync.dma_start(out_r[:, t, :], acc[:, t, :])

    nc.compile()
    return nc


# Row order of the gathered context: chunk hp = [even-core pair hp
# (heads 2hp,2hp+1), odd-core pair hp (heads 8+2hp, 8+2hp+1)].
_WO_ROW_ORDER = np.concatenate([
    np.concatenate([np.arange(128 * hp, 128 * hp + 128),
                    np.arange(512 + 128 * hp, 512 + 128 * hp + 128)])
    for hp in range(4)
])


def _bf16(a):
    import ml_dtypes
    return np.asarray(a, dtype=np.float32).astype(ml_dtypes.bfloat16)


def make_input_maps(x, Wq, Wk, Wv, Wo, bo):
    x = np.asarray(x, dtype=np.float32)
    Wq = np.asarray(Wq, dtype=np.float32)
    Wk = np.asarray(Wk, dtype=np.float32)
    Wv = np.asarray(Wv, dtype=np.float32)
    Wo = np.asarray(Wo, dtype=np.float32)[_WO_ROW_ORDER]
    bo = np.asarray(bo, dtype=np.float32)
    ins = []
    for c in range(8):
        b, g = c // 2, c % 2
        cols = slice(DPC * g, DPC * g + DPC)
        ins.append({
            "x": _bf16(x[b]),
            "wq": _bf16(Wq[:, cols]),
            "wk": _bf16(Wk[:, cols]),
            "wv": _bf16(Wv[:, cols]),
            "wo": _bf16(Wo[:, cols]),
            "bo": np.tile(bo[None, cols], (P, 1)).astype(np.float32),
        })
    return ins


def assemble(results):
    out = np.empty((B, S, D), np.float32)
    for c in range(8):
        b, g = c // 2, c % 2
        out[b, :, DPC * g:DPC * g + DPC] = results[c]["out"]
    return out


def kernel(x, Wq, Wk, Wv, Wo, bo):
    if "nc" not in _CACHE:
        _CACHE["nc"] = build_nc()
    nc = _CACHE["nc"]
    ins = make_input_maps(x, Wq, Wk, Wv, Wo, bo)
    res = run_bass_kernel_spmd(nc, ins, list(range(8)))
    return assemble(res.results)


# revision 36
# speedup vs baseline: 1.0122x; 1.0122x over previous
"""Multi-head causal attention (B=4, S=2048, D=1024, H=16) on 8 Trainium2
NeuronCores.

Sharding: core c handles batch c//2 and head-group c%2 (8 of 16 heads).
QKV weights are column-sharded per head-group; attention runs fully local.
Context vectors (bf16) are AllGathered pairwise in per-head-pair chunks
(the last chunk split by token range) so gathers overlap attention; each
core applies a column shard of the output projection.  Wo input rows are
pre-shuffled on the host to match the chunked gather's row order.

Single software-pipelined emission: projections for head-pair hp+1 and
output-projection partial sums for hp-1 are interleaved into hp's
attention inner loop, keeping PE busy under the ACT-bound exp stream.
The ctx matmuls lag the score matmuls by one group so the in-order PE
queue never stalls waiting on exp.  Scores for the even/odd head of a
pair are emitted adjacently with 64-partition operands so the PE
row-tiles them concurrently on HW.  Causal masking is applied post-exp
with gpsimd affine_select (zero fill) - no mask matmuls or bias tensors.
"""

from collections import deque

import numpy as np

import concourse.bass as bass
import concourse.tile as tile
from concourse import bacc, mybir
from concourse.bass import ts
from concourse.bass_utils import run_bass_kernel_spmd
from concourse.masks import make_identity

B, S, D, H, HD = 4, 2048, 1024, 16, 64
P = 128
DPC = 512                 # q/k/v dims per core (8 heads)
NT = S // P               # 16 token chunks
NKO = D // P              # 8 contraction chunks of the model dim
NQ = S // 512             # 4 q chunks of 512
NHP = DPC // P            # 4 local head pairs
F32 = mybir.dt.float32
FR = mybir.dt.float32r
BF16 = mybir.dt.bfloat16
EXP = mybir.ActivationFunctionType.Exp
MUL = mybir.AluOpType.mult
ADD = mybir.AluOpType.add
GROUPS = [[0, 1], [2, 3], [4, 5], [6, 7]]
# per head-pair gather pieces: (token offset, length, staged-after-chunk)
# stage=None means staged at the end of the head-pair's attention
GATHER_PIECES = {
    0: [(0, S, None)],
    1: [(0, S, None)],
    2: [(0, 1024, 1), (1024, 1024, None)],
    3: [(0, 1024, 1), (1024, 1024, None)],
}

_CACHE = {}


def build_nc():
    nc = bacc.Bacc("TRN2", target_bir_lowering=False, debug=False, num_devices=8)

    x_d = nc.declare_dram_parameter("x", [S, D], BF16, isOutput=False)
    wq_d = nc.declare_dram_parameter("wq", [D, DPC], BF16, isOutput=False)
    wk_d = nc.declare_dram_parameter("wk", [D, DPC], BF16, isOutput=False)
    wv_d = nc.declare_dram_parameter("wv", [D, DPC], BF16, isOutput=False)
    wo_d = nc.declare_dram_parameter("wo", [D, DPC], BF16, isOutput=False)
    bo_d = nc.declare_dram_parameter("bo", [P, DPC], F32, isOutput=False)
    out_d = nc.declare_dram_parameter("out", [S, DPC], F32, isOutput=True)

    with tile.TileContext(nc) as tc:
        with (
            tc.tile_pool(name="const", bufs=1) as cst,
            tc.tile_pool(name="big", bufs=1) as big,
            tc.tile_pool(name="dram", bufs=1, space="DRAM") as dramp,
            tc.tile_pool(name="cp", bufs=1) as cp,
            tc.tile_pool(name="ep", bufs=2) as ep,
            tc.tile_pool(name="xst", bufs=2) as xst,
            tc.tile_pool(name="cxf", bufs=2) as cxf,
            tc.tile_pool(name="cxd", bufs=2) as cxd,
            tc.tile_pool(name="psS", bufs=1, space="PSUM") as pss,
            tc.tile_pool(name="psC", bufs=1, space="PSUM") as psc,
            tc.tile_pool(name="psX", bufs=2, space="PSUM") as psx,
        ):
            ident = cst.tile([P, P], BF16)
            make_identity(nc, ident[:])
            ones_f = cst.tile([P, 64], F32)
            nc.vector.memset(ones_f[:], 1.0)
            ones_fr = cst.tile([P, 64], FR)
            nc.vector.tensor_copy(ones_fr[:], ones_f[:])
            bo_sb = cst.tile([P, DPC], F32)
            nc.sync.dma_start(bo_sb[:], bo_d[:])

            # persistent intermediates
            xT = big.tile([P, NKO, S], BF16)
            qT = big.tile([P, NHP, S], BF16)       # [dh, pair, tok]
            kT = big.tile([P, NHP, S], BF16)
            v_sb = big.tile([P, NT, 8, 65], BF16)  # [tok, chunk, head, dh+1]
            acc = big.tile([P, NT, DPC], F32)      # out-proj accumulator
            wk_sb = big.tile([P, NKO, DPC], BF16)
            wq_sb = big.tile([P, NKO, DPC], BF16)
            wv_sb = big.tile([P, NKO, DPC], BF16)
            wo_sb = big.tile([P, NKO, DPC], BF16)
            nc.gpsimd.dma_start(wv_sb[:], wv_d.rearrange("(ko p) n -> p ko n", p=P))
            nc.gpsimd.dma_start(wk_sb[:], wk_d.rearrange("(ko p) n -> p ko n", p=P))
            nc.gpsimd.dma_start(wq_sb[:], wq_d.rearrange("(ko p) n -> p ko n", p=P))
            nc.gpsimd.dma_start(wo_sb[:], wo_d.rearrange("(ko p) n -> p ko n", p=P))
            nc.vector.memset(v_sb[:, :, :, 64:65], 1.0)

            ctx_loc, ctx_ful = {}, {}
            for hp, pieces in GATHER_PIECES.items():
                for pi, (off, ln, _) in enumerate(pieces):
                    ctx_loc[(hp, pi)] = dramp.tile(
                        [P, ln], BF16, name=f"ctx_loc{hp}_{pi}")
                    ctx_ful[(hp, pi)] = dramp.tile(
                        [2 * P, ln], BF16, name=f"ctx_ful{hp}_{pi}")

            # ---- emission helpers ----
            def v_unit(t):
                def emit():
                    pv = psx.tile([P, DPC], F32, tag="x", name="pv")
                    for ko in range(NKO):
                        nc.tensor.matmul(
                            pv[:],
                            xT[:, ko, ts(t, P)],
                            wv_sb[:, ko, :],
                            start=(ko == 0),
                            stop=(ko == NKO - 1),
                        )
                    nc.vector.tensor_copy(
                        v_sb[:, t, :, 0:64],
                        pv[:].rearrange("p (h d) -> p h d", h=8),
                    )
                return emit

            def proj_unit(w_sb, outT, hp, n):
                def emit():
                    pq = psx.tile([P, DPC], F32, tag="x", name="pq")
                    for ko in range(NKO):
                        nc.tensor.matmul(
                            pq[:],
                            w_sb[:, ko, ts(hp, P)],
                            xT[:, ko, ts(n, 512)],
                            start=(ko == 0),
                            stop=(ko == NKO - 1),
                        )
                    nc.vector.tensor_copy(outT[:, hp, ts(n, 512)], pq[:])
                return emit

            def po_unit(hp, t, ctxf, t_off, first):
                def emit():
                    po = psx.tile([P, DPC], F32, tag="x", name="po")
                    for par in range(2):
                        nc.tensor.matmul(
                            po[:],
                            ctxf[:, par, ts(t - t_off, P)],
                            wo_sb[:, 2 * hp + par, :],
                            start=(par == 0),
                            stop=(par == 1),
                        )
                    if first:
                        nc.vector.tensor_tensor(acc[:, t, :], po[:], bo_sb[:], ADD)
                    else:
                        nc.vector.tensor_tensor(
                            acc[:, t, :], acc[:, t, :], po[:], ADD
                        )
                return emit

            pendn = deque()

            def normalize(pctx, ctx_dst, h01, c):
                # stage 1: reciprocal of the denominator row (DVE only)
                def stage1():
                    rec = cp.tile([P, 512], FR, tag=f"rec{h01}", name="rec")
                    with nc.allow_low_precision(reason="softmax recip"):
                        nc.vector.reciprocal(rec[64:65, :], pctx[64:65, :])

                    # stage 2: rank-1 broadcast via PE + scale (rec is ready)
                    def stage2():
                        pscl = psx.tile([P, DPC], F32, tag="x", name="pscl")
                        nc.tensor.matmul(
                            pscl[0:64, :], ones_fr[64:65, :], rec[64:65, :],
                            start=True, stop=True,
                        )
                        scl = cp.tile([64, 512], F32, tag=f"scl{h01}", name="scl")
                        nc.vector.tensor_copy(scl[:], pscl[0:64, :])
                        if h01 == 0:
                            nc.vector.tensor_tensor(
                                ctx_dst[0:64, ts(c, 512)], pctx[0:64, :],
                                scl[:], MUL,
                            )
                        else:
                            tmp = cp.tile([64, 512], BF16, tag="tmp", name="tmp")
                            nc.vector.tensor_tensor(
                                tmp[:], pctx[0:64, :], scl[:], MUL
                            )
                            nc.sync.dma_start(ctx_dst[64:128, ts(c, 512)], tmp[:])

                    pendn.append(stage2)
                return stage1

            def stage_piece(hp, pi, ctx_dst):
                off, ln, _ = GATHER_PIECES[hp][pi]
                nc.sync.dma_start(
                    ctx_loc[(hp, pi)][:], ctx_dst[:, off:off + ln]
                )
                nc.gpsimd.collective_compute(
                    "AllGather",
                    mybir.AluOpType.bypass,
                    replica_groups=GROUPS,
                    ins=[ctx_loc[(hp, pi)][:]],
                    outs=[ctx_ful[(hp, pi)][:]],
                )

            def attn_steps(hp, out):
                ctx_dst = cxd.tile([P, S], BF16, tag="ctxd", name=f"ctxd{hp}")
                out["ctx_dst"] = ctx_dst
                es = {}
                pctx_of = {}
                prev = []

                def ctx_mms(c, g):
                    nkb = 4 * c + 4
                    pctx = pctx_of[c]
                    for h01 in range(2):
                        e = es.pop((h01, g))
                        for dm in range(2):
                            m = 2 * g + dm
                            nc.tensor.matmul(
                                pctx[h01][0:65, :],
                                v_sb[:, m, 2 * hp + h01, 0:65],
                                e[:, dm, :],
                                start=(m == 0),
                                stop=(m == nkb - 1),
                            )

                def chunk_close(c):
                    ctx_mms(c, 2 * c + 1)
                    normalize(pctx_of[c][0], ctx_dst, 0, c)()
                    normalize(pctx_of[c][1], ctx_dst, 1, c)()

                staged = [pi for pi, p in enumerate(GATHER_PIECES[hp])
                          if p[2] is not None]
                for c in range(NQ):
                    pctx_of[c] = (
                        psc.tile([P, 512], F32, tag="c0", name="pctxE"),
                        psc.tile([P, 512], F32, tag="c1", name="pctxO"),
                    )
                    ngr = 2 * c + 2
                    for g in range(ngr):
                        yield c
                        sg = [
                            pss.tile([P, 2, 512], F32, tag=f"s{h}",
                                     name=f"sg{h}")
                            for h in range(2)
                        ]
                        # adjacent even/odd-head matmuls -> concurrent
                        # row-group tiles on HW (K=64 each)
                        for dm in range(2):
                            m = 2 * g + dm
                            for h01 in range(2):
                                off = 64 * h01
                                nc.tensor.matmul(
                                    sg[h01][:, dm, :],
                                    kT[off:off + 64, hp, ts(m, P)],
                                    qT[off:off + 64, hp, ts(c, 512)],
                                    start=True,
                                    stop=True,
                                )
                        for h01 in range(2):
                            e = ep.tile([P, 2, 512], BF16, tag=f"e{h01}",
                                        name=f"e{h01}")
                            es[(h01, g)] = e
                            nc.scalar.activation(
                                e[:], sg[h01][:], EXP, scale=0.125
                            )
                            # causal mask: zero e where key > query
                            for dm in range(2):
                                m = 2 * g + dm
                                dd = m - 4 * c
                                if dd >= 0:
                                    w = 128 * (dd + 1)
                                    nc.gpsimd.affine_select(
                                        out=e[:, dm, 0:w],
                                        in_=e[:, dm, 0:w],
                                        compare_op=mybir.AluOpType.is_ge,
                                        fill=0.0,
                                        base=-128 * dd,
                                        pattern=[[1, w]],
                                        channel_multiplier=-1,
                                    )
                        # deferred work rides behind the score/exp feed
                        for fn in prev:
                            fn()
                        prev = []
                        if g == 1 and staged and \
                                GATHER_PIECES[hp][staged[0]][2] == c - 1:
                            while pendn:
                                pendn.popleft()()
                            stage_piece(hp, staged.pop(0), ctx_dst)
                        drain(pendn, 2)
                        drain(pend_pe, 1)
                        if g == ngr - 1:
                            prev.append(lambda c=c: chunk_close(c))
                        else:
                            prev.append(lambda c=c, g=g: ctx_mms(c, g))
                for fn in prev:
                    fn()
                while pendn:
                    pendn.popleft()()

            # ---- phase A: transpose x; v + kq0 + early attn0 ride along ----
            pend_pe = deque()
            ctxf_tiles = {}

            def drain(q, n):
                for _ in range(min(n, len(q))):
                    q.popleft()()

            out_of = {0: {}}
            gen0 = attn_steps(0, out_of[0])
            next_c = [next(gen0)]

            def pull0(limit_c):
                if next_c[0] is not None and next_c[0] <= limit_c:
                    try:
                        next_c[0] = next(gen0)
                    except StopIteration:
                        next_c[0] = None
                    return True
                return False

            with nc.named_scope("phaseA"):
                for t in range(NT):
                    x_st = xst.tile([P, D], BF16, tag="x")
                    nc.sync.dma_start(
                        x_st[:], x_d.rearrange("(t p) d -> p t d", p=P)[:, t, :]
                    )
                    for half in range(2):
                        pt = psx.tile([P, 4, P], BF16, tag="x", name="pt")
                        for q in range(4):
                            nc.tensor.transpose(
                                pt[:, q, :], x_st[:, ts(4 * half + q, P)], ident[:]
                            )
                        if half == 1 and t > 0:
                            v_unit(t - 1)()
                            if t % 4 == 0:
                                n = t // 4 - 1
                                proj_unit(wk_sb, kT, 0, n)()
                                proj_unit(wq_sb, qT, 0, n)()
                        nc.vector.tensor_copy(
                            xT[:, 4 * half:4 * half + 4, ts(t, P)], pt[:]
                        )
                    if t >= 4:
                        pull0((t - 4) // 4)
                        if t >= 12:
                            pull0((t - 4) // 4)
                v_unit(NT - 1)()
                proj_unit(wk_sb, kT, 0, NQ - 1)()
                proj_unit(wq_sb, qT, 0, NQ - 1)()

            # ---- main interleaved loop ----
            for hp in range(NHP):
                with nc.named_scope(f"attn{hp}"):
                    if hp < NHP - 1:
                        for n in range(NQ):
                            pend_pe.append(proj_unit(wk_sb, kT, hp + 1, n))
                            pend_pe.append(proj_unit(wq_sb, qT, hp + 1, n))
                    if hp == 0:
                        while pull0(NQ):
                            pass
                    else:
                        out_of[hp] = {}
                        for _ in attn_steps(hp, out_of[hp]):
                            pass
                    ctx_dst = out_of[hp]["ctx_dst"]
                    drain(pend_pe, len(pend_pe))
                    if hp >= 1:
                        # gate: po batch for hp-1 cannot be scheduled before
                        # this attention's last ctx chunk is written
                        for pi, (off, ln, _) in enumerate(GATHER_PIECES[hp - 1]):
                            tag = ("cfS" if ln == S
                                   else f"cf{hp - 1}_{pi}")
                            cf = cxf.tile([P, 2, ln], BF16, tag=tag,
                                          name=f"ctxf{hp - 1}_{pi}", bufs=1)
                            nc.vector.tensor_copy(
                                cf[0:1, 0:1, 0:2], ctx_dst[0:1, S - 2:S]
                            )
                            nc.gpsimd.dma_start(
                                cf[:],
                                ctx_ful[(hp - 1, pi)].rearrange(
                                    "(h p) t -> p h t", p=P),
                            )
                            t0 = off // P
                            for t in range(t0, t0 + ln // P):
                                pend_pe.append(
                                    po_unit(hp - 1, t, cf, t0,
                                            first=(hp - 1 == 0))
                                )
                    # stage remaining (end) pieces for this hp
                    for pi, (off, ln, stage_c) in enumerate(GATHER_PIECES[hp]):
                        if stage_c is None:
                            stage_piece(hp, pi, ctx_dst)

            # ---- tail: hp3 out-proj pieces + write out ----
            with nc.named_scope("tail"):
                drain(pend_pe, len(pend_pe))
                ctxd3 = out_of[3]["ctx_dst"]
                out_r = out_d.rearrange("(t p) n -> p t n", p=P)
                gate_col = {0: S, 1: S}
                for pi, (off, ln, _) in enumerate(GATHER_PIECES[3]):
                    cf = cxf.tile([P, 2, ln], BF16, tag=f"cf3_{pi}",
                                  name=f"ctxf3_{pi}", bufs=1)
                    gc = gate_col[pi]
                    nc.vector.tensor_copy(
                        cf[0:1, 0:1, 0:2], ctxd3[0:1, gc - 2:gc]
                    )
                    nc.gpsimd.dma_start(
                        cf[:],
                        ctx_ful[(3, pi)].rearrange("(h p) t -> p h t", p=P),
                    )
                    t0 = off // P
                    for t in range(t0, t0 + ln // P):
                        po_unit(3, t, cf, t0, first=False)()
                        nc.sync.dma_start(out_r[:, t, :], acc[:, t, :])

    nc.compile()
    return nc


# Row order of the gathered context: chunk hp = [even-core pair hp
# (heads 2hp,2hp+1), odd-core pair hp (heads 8+2hp, 8+2hp+1)].
_WO_ROW_ORDER = np.concatenate([
    np.concatenate([np.arange(128 * hp, 128 * hp + 128),
                    np.arange(512 + 128 * hp, 512 + 128 * hp + 128)])
    for hp in range(4)
])


def _bf16(a):
    import ml_dtypes
    return np.asarray(a, dtype=np.float32).astype(ml_dtypes.bfloat16)


def make_input_maps(x, Wq, Wk, Wv, Wo, bo):
    x = np.asarray(x, dtype=np.float32)
    Wq = np.asarray(Wq, dtype=np.float32)
    Wk = np.asarray(Wk, dtype=np.float32)
    Wv = np.asarray(Wv, dtype=np.float32)
    Wo = np.asarray(Wo, dtype=np.float32)[_WO_ROW_ORDER]
    bo = np.asarray(bo, dtype=np.float32)
    ins = []
    for c in range(8):
        b, g = c // 2, c % 2
        cols = slice(DPC * g, DPC * g + DPC)
        ins.append({
            "x": _bf16(x[b]),
            "wq": _bf16(Wq[:, cols]),
            "wk": _bf16(Wk[:, cols]),
            "wv": _bf16(Wv[:, cols]),
            "wo": _bf16(Wo[:, cols]),
            "bo": np.tile(bo[None, cols], (P, 1)).astype(np.float32),
        })
    return ins


def assemble(results):
    out = np.empty((B, S, D), np.float32)
    for c in range(8):
        b, g = c // 2, c % 2
        out[b, :, DPC * g:DPC * g + DPC] = results[c]["out"]
    return out


def kernel(x, Wq, Wk, Wv, Wo, bo):
    if "nc" not in _CACHE:
        _CACHE["nc"] = build_nc()
    nc = _CACHE["nc"]
    ins = make_input_maps(x, Wq, Wk, Wv, Wo, bo)
    res = run_bass_kernel_spmd(nc, ins, list(range(8)))
    return assemble(res.results)
